# revision 40
# baseline (speedup 1.0000x reference)
"""Trainium2 Bass kernel for MIGAttention (topk token masking + GQA attention).

Shapes (hardcoded): B=4, N=2048, C=1024, H=16 heads, HKV=4 kv-heads, DH=64,
keep-ratio 0.7 -> k = 1433 selected tokens per batch row.

Sharding: 8 cores = (batch b in 0..3) x (query-half h in 0..1).  Each core
receives x[b].T with token columns rolled by h*1024 so that its own query
half always occupies columns 0..1023.

Key structure (on-device key AND query compaction):
 - x arrives as a bf16 hi/lo split so the router logits (4-pass bf16 matmul
   accumulated in fp32 PSUM) match fp32 within ~1e-6 -> exact top-k
   selection; all heavy matmuls run bf16.
 - The top-k threshold is found by 4 rounds of 128-ary refinement (emitted
   interleaved with the K/V-projection evictions so neither blocks the
   other on the in-order Vector stream); the selected token ids and their
   gate values are compacted with gpsimd sparse_gather (sentinel-padded via
   NaN-proof integer selects).
 - K,V (and Q) are projected token-major for all tokens (raw, ungated),
   stored to DRAM tables, and only the selected rows are fetched back with
   indirect DMA row-gathers (12x512B-row gathers for K|V, 6x2KB for my
   half's selected queries, zero rows behind the sentinel index).  K and Q
   are transposed to d-major with PE transpose-mode; gates are applied
   post-gather from the compacted gate lists.
 - Attention then runs over 12 key chunks x 768 query columns instead of
   16 x 1024: QK^T (zero-padded K=128 full-array bf16 matmuls -- sub-full-
   array matmuls run at half clock), exp on ScalarE (the bottleneck), att@V
   with the softmax denominator riding as a ones column of V.  Masked keys
   contribute exp(0)=1 to the reference softmax denominator, so a constant
   512 (= 615 masked keys - 103 sentinel rows) is added to the denominators;
   the sentinel query slot doubles as the shared output row of all masked
   queries (q=0 -> uniform attention), which the host broadcasts.
 - The host scatters the 768 compacted output rows back to their token
   positions using the device-computed index list (extra outputs idxq/nfq),
   falling back to a NumPy reference implementation if the selected-query
   count ever exceeded the padding (probability ~1e-6).
"""

import contextlib
import sys

import numpy as np

if "/opt/trn_rl_repo" not in sys.path:
    sys.path.insert(0, "/opt/trn_rl_repo")

import concourse.bass as bass  # noqa: F401
import concourse.bass_isa as bass_isa
import concourse.mybir as mybir
from concourse import bacc
from concourse.tile import TileContext

F32 = mybir.dt.float32
F32R = mybir.dt.float32r
BF16 = mybir.dt.bfloat16
I32 = mybir.dt.int32
I16 = mybir.dt.int16
U32 = mybir.dt.uint32
AF = mybir.ActivationFunctionType
ALU = mybir.AluOpType

B, N, C = 4, 2048, 1024
H, HKV, DH = 16, 4, 64
NQ = N // 2          # queries per core
KSEL = 1433          # max(1, int(N * 0.7))
NK2 = 1536           # padded selected-key count (12 chunks of 128)
KC2 = NK2 // 128     # 12
NQ2 = 768            # padded selected-query count (6 chunks of 128)
QC2 = NQ2 // 128     # 6
CC = C // 128        # contraction chunks (8)
QT_D = H * DH        # 1024
KV_D = HKV * DH      # 256
N_ROUNDS = 4         # topk threshold refinement rounds (8/128^4 = 3e-8)
LO0, W0 = -4.0, 8.0  # initial logit search interval (logit std ~0.65)
ONE_BITS = 0x3F800000  # fp32 bits of 1.0
INV_SQRT_DH = float(1.0 / np.sqrt(DH))


def _emit(nc, tc, ctx, io):
    xh, xl, wq, wkv, rw2, wo, out_d = (
        io["xh"], io["xl"], io["wq"], io["wkv"], io["rw2"], io["wo"],
        io["out"])

    # pools close LIFO: pa (refinement), pq (Q proj), pkv (KV proj), pg
    # (gather scratch), pm (masks), px (xh) -- created in reverse.
    const = ctx.enter_context(tc.tile_pool(name="const", bufs=1))
    small = ctx.enter_context(tc.tile_pool(name="small", bufs=1))
    big = ctx.enter_context(tc.tile_pool(name="big", bufs=1))
    dram = ctx.enter_context(tc.tile_pool(name="dram", bufs=1, space="DRAM"))

    px_ctx = contextlib.ExitStack()
    pm_ctx = contextlib.ExitStack()
    pg_ctx = contextlib.ExitStack()
    pkv_ctx = contextlib.ExitStack()
    pq_ctx = contextlib.ExitStack()
    pa_ctx = contextlib.ExitStack()
    px = px_ctx.enter_context(tc.tile_pool(name="px", bufs=1))
    psum1 = px_ctx.enter_context(tc.tile_pool(name="psum1", bufs=3, space="PSUM"))
    pm = pm_ctx.enter_context(tc.tile_pool(name="pm", bufs=1))
    pg = pg_ctx.enter_context(tc.tile_pool(name="pg", bufs=1))
    qtm_pool = pg_ctx.enter_context(tc.tile_pool(name="qtm", bufs=2))
    pkv = pkv_ctx.enter_context(tc.tile_pool(name="pkv", bufs=1))
    pq = pq_ctx.enter_context(tc.tile_pool(name="pq", bufs=1))
    pa = pa_ctx.enter_context(tc.tile_pool(name="pa", bufs=1))
    psum_r = pa_ctx.enter_context(tc.tile_pool(name="psum_r", bufs=1, space="PSUM"))

    # ---------------- constants ----------------
    ones2 = const.tile([2, 128], F32)
    nc.vector.memset(ones2, 1.0)
    iota128_i = const.tile([128, 1], I32)
    nc.gpsimd.iota(iota128_i, pattern=[[0, 1]], base=1, channel_multiplier=1)
    iota128 = const.tile([128, 1], F32)
    nc.vector.tensor_copy(iota128, iota128_i)
    sel8 = const.tile([16, CC, 128], F32R)
    nc.sync.dma_start(sel8, io["sel8"].bitcast(F32R))
    iota_w = const.tile([16, 128], I32)
    nc.gpsimd.iota(iota_w, pattern=[[16, 128]], base=0, channel_multiplier=1)
    iota_wf = const.tile([16, 128], F32)
    nc.vector.tensor_copy(iota_wf, iota_w)
    iota96 = const.tile([16, 96], I32)
    nc.gpsimd.iota(iota96, pattern=[[16, 96]], base=0, channel_multiplier=1)
    valid32 = const.tile([16, 96], I32)
    nc.vector.tensor_single_scalar(valid32, iota96, KSEL, op=ALU.is_lt)
    from concourse.masks import make_identity
    identity = const.tile([128, 128], BF16)
    make_identity(nc, identity)
    dum_out = const.tile([16, 1], F32)
    dum_nf = const.tile([1, 1], U32)
    # kt_z: zero-padded d-major K (slot j holds kv-heads 2j/2j+1; parity p
    # keeps rows [64p,64p+64) live, the partner half zero)
    kt_z = big.tile([128, 2, 2, NK2], BF16)
    nc.vector.memset(kt_z[0:64, 1, :, :], 0.0)
    nc.vector.memset(kt_z[64:128, 0, :, :], 0.0)
    # v_sb[t, kc, hk, 0:64]=V, col 64 = ones (softmax denominator), 65: zero
    v_sb = big.tile([128, KC2, HKV, 128], BF16)
    nc.vector.memset(v_sb[:, :, :, 64:128], 0.0)
    nc.vector.memset(v_sb[:, :, :, 64:65], 1.0)

    # ---------------- loads (split DMAs to spread across queues) ---------
    xh_sb = px.tile([128, CC, N], BF16)
    for cc in range(CC):
        for hf in range(2):
            nc.sync.dma_start(
                xh_sb[:, cc, hf * 1024:(hf + 1) * 1024],
                xh[cc * 128:(cc + 1) * 128, hf * 1024:(hf + 1) * 1024])
    rw_sb = pa.tile([128, CC, 128], BF16)
    for cc in range(CC):
        nc.sync.dma_start(rw_sb[:, cc, :], rw2[cc * 128:(cc + 1) * 128, :])
    xl_sb = pa.tile([128, CC, N], BF16)
    for cc in range(CC):
        for hf in range(2):
            nc.sync.dma_start(xl_sb[:, cc, hf * 1024:(hf + 1) * 1024],
                              xl[cc * 128:(cc + 1) * 128,
                                 hf * 1024:(hf + 1) * 1024])
    wkv_sb = pkv.tile([128, CC, 512], BF16)
    for cc in range(CC):
        nc.sync.dma_start(wkv_sb[:, cc, :], wkv[cc * 128:(cc + 1) * 128, :])
    wq_sb = pq.tile([128, CC, QT_D], BF16)
    for cc in range(CC):
        nc.sync.dma_start(wq_sb[:, cc, :], wq[cc * 128:(cc + 1) * 128, :])

    # ---------------- router: logits = (hi+lo) @ (rw_hi+rw_lo) ------------
    # M=128 stationary (126 zero cols) -> full-array matmuls stay at 2.4GHz
    rps = [psum_r.tile([128, 512], F32, tag=f"router_ps{g}",
                       name=f"router_ps{g}") for g in range(4)]
    for cc in range(CC):
        for g in range(4):
            nc.tensor.matmul(
                rps[g], rw_sb[:, cc, :], xh_sb[:, cc, g * 512:(g + 1) * 512],
                start=(cc == 0), stop=False)
    for cc in range(CC):
        for g in range(4):
            nc.tensor.matmul(
                rps[g], rw_sb[:, cc, :], xl_sb[:, cc, g * 512:(g + 1) * 512],
                start=False, stop=(cc == CC - 1))
    logits2 = pa.tile([2, N], F32)
    for g in range(4):
        nc.vector.tensor_copy(logits2[:, g * 512:(g + 1) * 512],
                              rps[g][0:2, :])
    lrep = pa.tile([128, N], F32)
    for g in range(4):
        ps = psum_r.tile([128, 512], F32, tag="bcast_ps")
        nc.tensor.matmul(ps, ones2, logits2[:, g * 512:(g + 1) * 512],
                         start=True, stop=True)
        nc.vector.tensor_copy(lrep[:, g * 512:(g + 1) * 512], ps)

    # ---------------- K/V projection (token-major, raw/ungated) ----------
    # kv_tm[t, i, 0:256]=K row, [256:512]=V row -> DRAM table (one DMA per
    # 128-token chunk to spread queues); the gate is applied post-gather.
    # The topk refinement rounds are EMITTED INTERLEAVED with the evictions
    # so the refinement's small DVE ops don't queue behind all 16 evicts on
    # the in-order Vector stream (nor the evicts behind the refinement).
    lo = small.tile([128, 1], F32)
    nc.vector.memset(lo, LO0)
    neg_edges = small.tile([128, 1], F32)
    acc = small.tile([128, 1], F32)
    sel = small.tile([128, 1], F32)
    ssum = small.tile([128, 1], F32)
    # the Sign output is never read: write it over xl (dead after router)
    sign_scr = xl_sb[:, 0, :]
    thr_acc = float(2 * KSEL - N)

    def emit_round(r):
        wstep = W0 / (128.0 ** (r + 1))
        nc.vector.scalar_tensor_tensor(
            neg_edges, iota128, -wstep, lo, op0=ALU.mult, op1=ALU.subtract)
        nc.scalar.activation(sign_scr, lrep, AF.Sign, bias=neg_edges,
                             scale=1.0, accum_out=acc)
        nc.vector.tensor_single_scalar(sel, acc, thr_acc, op=ALU.is_ge)
        nc.gpsimd.partition_all_reduce(ssum, sel, channels=128,
                                       reduce_op=bass_isa.ReduceOp.add)
        nc.vector.scalar_tensor_tensor(
            lo, ssum, wstep, lo, op0=ALU.mult, op1=ALU.add)

    kv_tm = pg.tile([128, 16, 512], BF16, tag="tm")
    kv_dram = dram.tile([N + 1, 512], BF16)
    zrow = pg.tile([1, 512], BF16)
    nc.vector.memset(zrow, 0.0)
    nc.sync.dma_start(kv_dram[N:N + 1, :], zrow)
    next_round = 0
    for i in range(16):
        ps = psum1.tile([128, 512], F32, tag="proj_ps", name=f"kv_ps{i}")
        for cc in range(CC):
            nc.tensor.matmul(
                ps, xh_sb[:, cc, i * 128:(i + 1) * 128], wkv_sb[:, cc, :],
                start=(cc == 0), stop=(cc == CC - 1))
        nc.vector.tensor_copy(kv_tm[:, i, :], ps)
        nc.sync.dma_start(
            kv_dram[i * 128:(i + 1) * 128, :].rearrange("(a p) c -> p a c",
                                                        p=128),
            kv_tm[:, i:i + 1, :])
        if i % 3 == 2 and next_round < N_ROUNDS:
            emit_round(next_round)
            next_round += 1
    while next_round < N_ROUNDS:
        emit_round(next_round)
        next_round += 1
    # dummy sparse_gather whose input depends on the last round: the
    # library load overlaps the m/cand step instead of running at t=0
    dum_in = pg.tile([16, 1], F32)
    nc.vector.scalar_tensor_tensor(dum_in, lo[0:16, :], -1.0,
                                   lo[0:16, :], op0=ALU.mult,
                                   op1=ALU.subtract)
    nc.gpsimd.sparse_gather(dum_out, dum_in, num_found=dum_nf)

    # m = (logit > lo) * sigmoid(logit); row 0 is enough for the compaction
    # sigmoid row parked in xl chunks 2-3 (dead after router, base 0)
    grep_row = xl_sb[0:1, 2:4, :].rearrange("p a b -> p (a b)").bitcast(F32)
    nc.scalar.activation(grep_row, lrep[0:1, :], AF.Sigmoid)
    m_row = logits2[0:1, :]
    nc.vector.scalar_tensor_tensor(
        m_row, lrep[0:1, :], lo[0:1, :], grep_row,
        op0=ALU.is_gt, op1=ALU.mult)

    # ---------------- compaction: index + gate lists ----------------
    m_dram = dram.tile([N], F32)
    nc.sync.dma_start(m_dram, m_row)
    m_w = pg.tile([16, 128], F32)
    nc.sync.dma_start(m_w, m_dram.rearrange("(f p) -> p f", p=16))
    selw = pg.tile([16, 128], F32)
    nc.vector.tensor_single_scalar(selw, m_w, 0.0, op=ALU.is_gt)
    # cand_idx = sel*(iota+1)-1 ; cand_gate = sel*(m+1)-1  (=-1 if unselected)
    cand_i = pg.tile([16, 128], F32)
    t1 = pg.tile([16, 128], F32)
    nc.vector.tensor_scalar(t1, iota_wf, 1.0, None, op0=ALU.add)
    nc.vector.tensor_tensor(cand_i, selw, t1, op=ALU.mult)
    nc.vector.tensor_scalar(cand_i, cand_i, 1.0, None, op0=ALU.subtract)
    cand_g = pg.tile([16, 128], F32)
    nc.vector.tensor_scalar(t1, m_w, 1.0, None, op0=ALU.add)
    nc.vector.tensor_tensor(cand_g, selw, t1, op=ALU.mult)
    nc.vector.tensor_scalar(cand_g, cand_g, 1.0, None, op0=ALU.subtract)
    # idx chain first: it alone gates the indirect gathers.  The gate chain
    # (second sparse_gather etc.) is only needed for the post-gather gate
    # multiplies and runs in parallel behind it.
    idx_f = pg.tile([16, 96], F32)
    nf = pg.tile([1, 1], U32)
    nc.gpsimd.sparse_gather(idx_f, cand_i, num_found=nf)
    # positions >= KSEL -> idx = sentinel N (zero row); integer selects so
    # sparse_gather tail garbage (even NaN bits) is annihilated
    idx32 = pg.tile([16, 96], I32)
    nc.vector.tensor_copy(idx32, idx_f)
    nc.vector.tensor_scalar(idx32, idx32, N, None, op0=ALU.subtract)
    nc.vector.tensor_tensor(idx32, valid32, idx32, op=ALU.mult)
    nc.vector.tensor_scalar(idx32, idx32, N, None, op0=ALU.add)
    idx_dram = dram.tile([NK2], I32)
    nc.sync.dma_start(idx_dram.rearrange("(f p) -> p f", p=16), idx32)
    idx_pc = pg.tile([128, KC2], I32)
    nc.sync.dma_start(idx_pc, idx_dram.rearrange("(c p) -> p c", p=128))

    # ---------------- Q projection (token-major, raw) -> DRAM table ------
    q_dram = dram.tile([NQ + 1, QT_D], BF16)
    for zh in range(2):
        nc.sync.dma_start(q_dram[NQ:NQ + 1, zh * 512:(zh + 1) * 512], zrow)
    for tt in range(8):
        q_tm = qtm_pool.tile([128, QT_D], BF16, tag="qtm", name=f"qtm{tt}")
        for qh in range(2):
            ps = psum1.tile([128, 512], F32, tag="proj_ps",
                            name=f"q_ps{tt}_{qh}")
            for cc in range(CC):
                nc.tensor.matmul(
                    ps, xh_sb[:, cc, tt * 128:(tt + 1) * 128],
                    wq_sb[:, cc, qh * 512:(qh + 1) * 512],
                    start=(cc == 0), stop=(cc == CC - 1))
            nc.vector.tensor_copy(q_tm[:, qh * 512:(qh + 1) * 512], ps)
        nc.sync.dma_start(
            q_dram[tt * 128:(tt + 1) * 128, :], q_tm)
    # ---------------- gather the selected K/V rows ----------------
    pa_ctx.close()  # frees the router PSUM banks for the transpose pool
    ptr_ctx = contextlib.ExitStack()
    ptr_psum = ptr_ctx.enter_context(
        tc.tile_pool(name="ptr", bufs=4, space="PSUM"))
    # remaining sparse_gathers grouped here: one gpsimd library window for
    # all four, then one reload for all 18 indirect gathers
    gat_f = pg.tile([16, 96], F32)
    nf2 = pg.tile([1, 1], U32)
    nc.gpsimd.sparse_gather(gat_f, cand_g, num_found=nf2)
    candq_i = pg.tile([16, 64], F32)
    candq_g = pg.tile([16, 64], F32)
    nc.vector.tensor_scalar(t1[:, 0:64], iota_wf[:, 0:64], 1.0, None,
                            op0=ALU.add)
    nc.vector.tensor_tensor(candq_i, selw[:, 0:64], t1[:, 0:64], op=ALU.mult)
    nc.vector.tensor_scalar(candq_i, candq_i, 1.0, None, op0=ALU.subtract)
    nc.vector.tensor_scalar(t1[:, 0:64], m_w[:, 0:64], 1.0, None, op0=ALU.add)
    nc.vector.tensor_tensor(candq_g, selw[:, 0:64], t1[:, 0:64], op=ALU.mult)
    nc.vector.tensor_scalar(candq_g, candq_g, 1.0, None, op0=ALU.subtract)
    idxq_f = pg.tile([16, 48], F32)
    nfq = pg.tile([1, 1], U32)
    nc.gpsimd.sparse_gather(idxq_f, candq_i, num_found=nfq)
    gatq_f = pg.tile([16, 48], F32)
    nfq2 = pg.tile([1, 1], U32)
    nc.gpsimd.sparse_gather(gatq_f, candq_g, num_found=nfq2)
    # q idx/gate ints + roundtrips (DVE/DMA only)
    idxq32 = pg.tile([16, 48], I32)
    nc.vector.tensor_copy(idxq32, idxq_f)
    nc.vector.tensor_scalar(idxq32, idxq32, NQ, 0, op0=ALU.min, op1=ALU.max)
    validq = pg.tile([16, 48], I32)
    nc.vector.tensor_single_scalar(validq, iota96[:, 0:48], NQ2 - 1,
                                   op=ALU.is_lt)
    nc.vector.tensor_scalar(idxq32, idxq32, NQ, None, op0=ALU.subtract)
    nc.vector.tensor_tensor(idxq32, validq, idxq32, op=ALU.mult)
    nc.vector.tensor_scalar(idxq32, idxq32, NQ, None, op0=ALU.add)
    gatq32 = pg.tile([16, 48], I32)
    nc.vector.tensor_copy(gatq32, gatq_f.bitcast(I32))
    nc.vector.tensor_scalar(gatq32, gatq32, ONE_BITS, None, op0=ALU.subtract)
    nc.vector.tensor_tensor(gatq32, validq, gatq32, op=ALU.mult)
    nc.vector.tensor_scalar(gatq32, gatq32, ONE_BITS, None, op0=ALU.add)
    idxq_dram = dram.tile([NQ2], I32)
    nc.sync.dma_start(idxq_dram.rearrange("(f p) -> p f", p=16), idxq32)
    gateq_dram = dram.tile([NQ2], F32)
    nc.sync.dma_start(gateq_dram.rearrange("(f p) -> p f", p=16),
                      gatq32.bitcast(F32))
    idxq_pc = pg.tile([128, QC2], I32)
    nc.sync.dma_start(idxq_pc, idxq_dram.rearrange("(c p) -> p c", p=128))
    gateq_row = pg.tile([1, NQ2], F32)
    nc.sync.dma_start(gateq_row, gateq_dram)
    nc.sync.dma_start(io["idxq"], idxq32)
    nc.sync.dma_start(io["nfq"], nfq.bitcast(I32))
    # all indirect gathers (single mlp-library window); q rows first so the
    # q transpose/gate chain (PE/DVE) overlaps the kv gathers (gpsimd)
    q_g = pg.tile([128, 8, QT_D], BF16, tag="tm")
    for ch in range(QC2):
        nc.gpsimd.indirect_dma_start(
            out=q_g[:, ch, :],
            out_offset=None,
            in_=q_dram,
            in_offset=bass.IndirectOffsetOnAxis(ap=idxq_pc[:, ch:ch + 1],
                                                axis=0),
        )
    kv_g = pg.tile([128, KC2, 512], BF16)
    for ch in range(KC2):
        nc.gpsimd.indirect_dma_start(
            out=kv_g[:, ch, :],
            out_offset=None,
            in_=kv_dram,
            in_offset=bass.IndirectOffsetOnAxis(ap=idx_pc[:, ch:ch + 1],
                                                axis=0),
        )
    # k gate ints (DVE)
    gat32 = pg.tile([16, 96], I32)
    nc.vector.tensor_copy(gat32, gat_f.bitcast(I32))
    nc.vector.tensor_scalar(gat32, gat32, ONE_BITS, None, op0=ALU.subtract)
    nc.vector.tensor_tensor(gat32, valid32, gat32, op=ALU.mult)
    nc.vector.tensor_scalar(gat32, gat32, ONE_BITS, None, op0=ALU.add)
    gate_dram = dram.tile([NK2], F32)
    nc.sync.dma_start(gate_dram.rearrange("(f p) -> p f", p=16),
                      gat32.bitcast(F32))
    gate_g = pg.tile([128, KC2], F32)
    nc.sync.dma_start(gate_g, gate_dram.rearrange("(c p) -> p c", p=128))
    gate_row = pg.tile([1, NK2], F32)
    nc.sync.dma_start(gate_row, gate_dram)
    grep_sel = pg.tile([128, NK2], BF16, tag="gsel")
    for g in range(NK2 // 512):
        ps = psum1.tile([128, 512], F32, tag="proj_ps", name=f"gb{g}")
        nc.tensor.matmul(ps, ones2[0:1, :].bitcast(F32),
                         gate_row[:, g * 512:(g + 1) * 512],
                         start=True, stop=True)
        nc.vector.tensor_copy(grep_sel[:, g * 512:(g + 1) * 512], ps)
    qt_sb = big.tile([128, H // 2, NQ2], BF16)
    for ch in range(QC2):
        for sl8 in range(8):
            trp = ptr_psum.tile([128, 128], BF16, tag="trp",
                                name=f"trq{ch}_{sl8}")
            nc.tensor.transpose(
                trp, q_g[:, ch, sl8 * 128:(sl8 + 1) * 128], identity)
            nc.vector.tensor_copy(qt_sb[:, sl8, ch * 128:(ch + 1) * 128], trp)
    # gate the queries: qg_rep = broadcast(gateq_row) then qt *= qg_rep
    qg_rep = pg.tile([128, NQ2], BF16)
    for gi, (qs, qw) in enumerate([(0, 512), (512, 256)]):
        ps = psum1.tile([128, 512], F32, tag="proj_ps", name=f"qgb{gi}")
        nc.tensor.matmul(ps[:, 0:qw], ones2[0:1, :].bitcast(F32),
                         gateq_row[:, qs:qs + qw], start=True, stop=True)
        nc.vector.tensor_copy(qg_rep[:, qs:qs + qw], ps[:, 0:qw])
    for j in range(H // 2):
        nc.vector.tensor_tensor(qt_sb[:, j, :], qt_sb[:, j, :], qg_rep,
                                op=ALU.mult)
    # K -> d-major via PE transpose-mode (PE is idle here), split into the
    # kt_z parity halves
    for ch in range(KC2):
        for s in range(2):
            trp = ptr_psum.tile([128, 128], BF16, tag="trp",
                                name=f"trp{ch}_{s}")
            nc.tensor.transpose(trp, kv_g[:, ch, s * 128:(s + 1) * 128],
                                identity)
            cs = slice(ch * 128, (ch + 1) * 128)
            nc.vector.tensor_copy(kt_z[0:64, 0, s, cs], trp[0:64, :])
            nc.vector.tensor_copy(kt_z[64:128, 1, s, cs], trp[64:128, :])
    # gate: kt columns (d-major) and v rows (token-major)
    for par in range(2):
        for s in range(2):
            nc.vector.tensor_tensor(kt_z[:, par, s, :], kt_z[:, par, s, :],
                                    grep_sel, op=ALU.mult)
    for ch in range(KC2):
        nc.vector.tensor_scalar(
            v_sb[:, ch, :, 0:64],
            kv_g[:, ch, 256:512].rearrange("p (h e) -> p h e", e=64),
            gate_g[:, ch:ch + 1], None, op0=ALU.mult)

    ptr_ctx.close()
    pq_ctx.close()
    pkv_ctx.close()
    pg_ctx.close()
    pm_ctx.close()
    px_ctx.close()

    # ---------------- phase 2: attention ----------------
    ph2_ctx = contextlib.ExitStack()
    ph2 = ph2_ctx.enter_context(tc.tile_pool(name="ph2", bufs=1))
    wo_sb = ph2.tile([128, CC, C], BF16)
    for cc in range(CC):
        nc.sync.dma_start(wo_sb[:, cc, :], wo[cc * 128:(cc + 1) * 128, :])

    patt_ctx = contextlib.ExitStack()
    scr_pool = patt_ctx.enter_context(tc.tile_pool(name="scr_pool", bufs=2))
    p_pool = patt_ctx.enter_context(tc.tile_pool(name="p_pool", bufs=2))
    lg_pool = patt_ctx.enter_context(
        tc.tile_pool(name="lg_pool", bufs=2, space="PSUM"))
    att_pool = patt_ctx.enter_context(
        tc.tile_pool(name="att_pool", bufs=1, space="PSUM"))
    oT_sb = ph2.tile([128, CC, NQ2], BF16)
    denom_sb = ph2.tile([16, NQ2], F32)

    KQ = 2  # key chunks per P buffer
    QG = [(0, 512), (512, 256)]  # query column groups (NQ2 = 768)
    pair_heads = [(ha, ha + 4) for ha in (0, 1, 2, 3, 8, 9, 10, 11)]
    for hp, pair in enumerate(pair_heads):
        att_ps = [att_pool.tile([128, NQ2], F32, tag=f"att{m}", name=f"att{hp}_{m}")
                  for m in range(2)]
        pend = []
        for quarter in range(KC2 // KQ):
            p_t = p_pool.tile([128, KQ, 2 * NQ2], BF16, tag="p_t",
                              name=f"p_{hp}_{quarter}")
            for kci in range(KQ):
                kc = quarter * KQ + kci
                lg = [lg_pool.tile([128, NQ2], F32, tag="lg",
                                   name=f"lg{hp}_{kc}_{m2}") for m2 in range(2)]
                for qs, qw in QG:
                    for m in range(2):
                        h = pair[m]
                        hkv = h // 4
                        jq = (h % 4) + 4 * (h // 8)
                        nc.tensor.matmul(
                            lg[m][:, qs:qs + qw],
                            kt_z[:, hkv % 2, hkv // 2,
                                 kc * 128:(kc + 1) * 128],
                            qt_sb[:, jq, qs:qs + qw],
                            start=True, stop=True)
                for m in range(2):
                    nc.scalar.activation(
                        p_t[:, kci, m * NQ2:(m + 1) * NQ2], lg[m], AF.Exp,
                        scale=INV_SQRT_DH)
                for f in pend:
                    f()
                pend = []

                def attv(p_t=p_t, kci=kci, kc=kc):
                    for m in range(2):
                        hk = pair[m] // 4
                        for qs, qw in QG:
                            nc.tensor.matmul(
                                att_ps[m][:, qs:qs + qw],
                                v_sb[:, kc, hk, :],
                                p_t[:, kci, m * NQ2 + qs:m * NQ2 + qs + qw],
                                start=(kc == 0), stop=(kc == KC2 - 1))

                pend.append(attv)
        for f in pend:
            f()
        # fast evict: one [65, NQ2] copy frees the att psum for the next
        # pair; denom row DMA + bf16 conversion read the sbuf scratch.
        for m in range(2):
            h = pair[m]
            scr65 = scr_pool.tile([65, NQ2], F32R, tag="scr65",
                                  name=f"scr65_{hp}_{m}")
            nc.vector.tensor_copy(scr65, att_ps[m][0:65, :])
            nc.sync.dma_start(denom_sb[h:h + 1, :],
                              scr65[64:65, :].bitcast(F32))
            if h % 2 == 0:
                nc.vector.tensor_copy(oT_sb[0:64, h // 2, :], scr65[0:64, :])
            else:
                scr64 = scr_pool.tile([64, NQ2], BF16, tag="scr64",
                                      name=f"scr64_{hp}_{m}")
                nc.vector.tensor_copy(scr64, scr65[0:64, :])
                nc.sync.dma_start(oT_sb[64:128, h // 2, :], scr64)

    # denominators: add the 512 missing exp(0)=1 contributions (615 masked
    # keys in the reference minus 103 zero-padding sentinel rows on device),
    # then reciprocal + broadcast + scale
    nc.vector.tensor_scalar(denom_sb, denom_sb, 512.0, None, op0=ALU.add)
    rec16 = ph2.tile([16, NQ2], F32R)
    rec16_f = ph2.tile([16, NQ2], F32)
    with nc.allow_low_precision(reason="2e-5 rel err << output tolerance"):
        nc.vector.reciprocal_approx_fast(out=rec16_f, in_=denom_sb)
    nc.vector.tensor_copy(rec16, rec16_f)
    ph3_ctx = contextlib.ExitStack()
    out_pool = ph3_ctx.enter_context(tc.tile_pool(name="out_pool", bufs=2))
    for gi, (qs, qw) in enumerate([(0, 512), (512, 256)]):
        for dd in range(CC):
            bps = lg_pool.tile([128, 512], F32, tag="lg", name=f"bps{dd}_{gi}")
            nc.tensor.matmul(
                bps[:, 0:qw], sel8[:, dd, :], rec16[:, qs:qs + qw],
                start=True, stop=True)
            nc.vector.tensor_tensor(
                oT_sb[:, dd, qs:qs + qw], oT_sb[:, dd, qs:qs + qw],
                bps[:, 0:qw], op=ALU.mult)
        for tt in range(qs // 128, (qs + qw) // 128):
            out_sb = out_pool.tile([128, C], F32, tag="out_sb",
                                   name=f"out_sb{tt}")
            for og in range(C // 512):
                ps = att_pool.tile([128, 512], F32, tag=f"att{og}",
                                   name=f"out_ps{tt}_{og}")
                for dd in range(CC):
                    nc.tensor.matmul(
                        ps, oT_sb[:, dd, tt * 128:(tt + 1) * 128],
                        wo_sb[:, dd, og * 512:(og + 1) * 512],
                        start=(dd == 0), stop=(dd == CC - 1))
                nc.scalar.copy(out_sb[:, og * 512:(og + 1) * 512], ps)
            for hf in range(2):
                nc.sync.dma_start(
                    out_d[tt * 128:(tt + 1) * 128, hf * 512:(hf + 1) * 512],
                    out_sb[:, hf * 512:(hf + 1) * 512])
    ph3_ctx.close()
    patt_ctx.close()
    ph2_ctx.close()


_NC = None


def build_program():
    global _NC
    if _NC is not None:
        return _NC
    from contextlib import ExitStack

    nc = bacc.Bacc("TRN2", target_bir_lowering=False, debug=False, num_devices=8)
    io = {
        "xh": nc.dram_tensor("xh", (C, N), BF16, kind="ExternalInput").ap(),
        "xl": nc.dram_tensor("xl", (C, N), BF16, kind="ExternalInput").ap(),
        "wq": nc.dram_tensor("wq", (C, QT_D), BF16, kind="ExternalInput").ap(),
        "wkv": nc.dram_tensor("wkv", (C, 512), BF16, kind="ExternalInput").ap(),
        "rw2": nc.dram_tensor("rw2", (C, 128), BF16,
                              kind="ExternalInput").ap(),
        "wo": nc.dram_tensor("wo", (C, C), BF16, kind="ExternalInput").ap(),
        "sel8": nc.dram_tensor("sel8", (16, CC, 128), F32,
                               kind="ExternalInput").ap(),
        "out": nc.dram_tensor("out", (NQ2, C), F32,
                              kind="ExternalOutput").ap(),
        "idxq": nc.dram_tensor("idxq", (16, 48), I32,
                               kind="ExternalOutput").ap(),
        "nfq": nc.dram_tensor("nfq", (1,), I32, kind="ExternalOutput").ap(),
    }
    with TileContext(nc) as tc:
        with ExitStack() as ctx:
            _emit(nc, tc, ctx, io)
    nc.compile()
    _NC = nc
    return nc


def _permute_wq(wq):
    """Column-permute wq so QT slot j's 128 cols = heads (ha, ha+4) contig."""
    wq = np.asarray(wq, np.float32).reshape(C, H, DH)
    order = []
    for j in range(H // 2):
        ha = j if j < 4 else j + 4
        order += [ha, ha + 4]
    return np.ascontiguousarray(wq[:, order, :].reshape(C, H * DH))


def make_in_maps(x, router_w, wq, wk, wv, wo):
    import ml_dtypes

    bf16 = ml_dtypes.bfloat16
    wq = _permute_wq(wq).astype(bf16)
    rw = np.asarray(router_w, np.float32)
    rw_hi = rw.astype(bf16)
    rw_lo = (rw - rw_hi.astype(np.float32)).astype(bf16)
    rw2 = np.zeros((C, 128), np.float32).astype(bf16)
    rw2[:, 0:1] = rw_hi
    rw2[:, 1:2] = rw_lo
    rw2 = np.ascontiguousarray(rw2)
    wkv = np.ascontiguousarray(
        np.concatenate([np.asarray(wk, np.float32),
                        np.asarray(wv, np.float32)], axis=1).astype(bf16))
    wo_b = np.asarray(wo, np.float32).astype(bf16)
    sel8 = np.zeros((16, CC, 128), np.float32)
    for dd in range(CC):
        for p in range(128):
            sel8[2 * dd + p // 64, dd, p] = 1.0
    in_maps = []
    for core in range(8):
        b, h = core // 2, core % 2
        xT_core = np.ascontiguousarray(
            np.roll(np.asarray(x[b], np.float32).T, -h * NQ, axis=1))
        xh_core = xT_core.astype(bf16)
        xl_core = (xT_core - xh_core.astype(np.float32)).astype(bf16)
        in_maps.append({
            "xh": np.ascontiguousarray(xh_core),
            "xl": np.ascontiguousarray(xl_core),
            "sel8": sel8,
            "wq": np.ascontiguousarray(wq),
            "wkv": wkv,
            "rw2": rw2,
            "wo": np.ascontiguousarray(wo_b),
        })
    return in_maps


def _numpy_fallback(x, router_w, router_b, wq, bq, wk, bk, wv, bv, wo, bo):
    x = np.asarray(x, np.float32)
    gate = 1.0 / (1.0 + np.exp(-(x @ router_w + router_b)))
    xg = x * gate
    scores = gate[..., 0]
    idx = np.argsort(-scores, axis=-1, kind="stable")[:, :KSEL]
    mask = np.zeros((x.shape[0], x.shape[1]), np.float32)
    np.put_along_axis(mask, idx, 1.0, axis=1)
    xg = xg * mask[..., None]
    q = (xg @ wq + bq).reshape(B, N, H, DH)
    kk = np.repeat((xg @ wk + bk).reshape(B, N, HKV, DH), H // HKV, axis=2)
    v = np.repeat((xg @ wv + bv).reshape(B, N, HKV, DH), H // HKV, axis=2)
    att = np.einsum("bqhd,bkhd->bhqk", q, kk) / np.float32(np.sqrt(DH))
    att = att - att.max(-1, keepdims=True)
    att = np.exp(att)
    att = att / att.sum(-1, keepdims=True)
    o = np.einsum("bhqk,bkhd->bqhd", att, v).reshape(B, N, C)
    return (o @ wo + bo).astype(np.float32)


def kernel(x, router_w, router_b, wq, bq, wk, bk, wv, bv, wo, bo):
    x = np.asarray(x)
    biases = [router_b, bq, bk, bv, bo]
    if any(float(np.abs(np.asarray(t)).max()) != 0.0 for t in biases):
        return _numpy_fallback(x, router_w, router_b, wq, bq, wk, bk, wv, bv,
                               wo, bo)

    from concourse import bass_utils

    nc = build_program()
    in_maps = make_in_maps(x, router_w, wq, wk, wv, wo)
    res = bass_utils.run_bass_kernel_spmd(nc, in_maps, core_ids=list(range(8)))
    out = np.empty((B, N, C), np.float32)
    for core in range(8):
        b, h = core // 2, core % 2
        r = res.results[core]
        nf = int(np.asarray(r["nfq"]).view(np.int32)[0])
        idxq = np.ascontiguousarray(np.asarray(r["idxq"]).T).reshape(-1)
        rows = np.asarray(r["out"])
        if not (0 < nf <= NQ2 - 1):
            return _numpy_fallback(x, router_w, router_b, wq, bq, wk, bk,
                                   wv, bv, wo, bo)
        ids = idxq[:nf]
        if ids.min() < 0 or ids.max() >= NQ:
            return _numpy_fallback(x, router_w, router_b, wq, bq, wk, bk,
                                   wv, bv, wo, bo)
        # non-selected queries all share the sentinel (zero-q) row's output
        block = np.repeat(rows[NQ2 - 1:NQ2, :], NQ, axis=0)
        block[ids] = rows[:nf]
        out[b, h * NQ:(h + 1) * NQ, :] = block
    return out


# revision 41
# speedup vs baseline: 1.0113x; 1.0113x over previous
"""Trainium2 Bass kernel for MIGAttention (topk token masking + GQA attention).

Shapes (hardcoded): B=4, N=2048, C=1024, H=16 heads, HKV=4 kv-heads, DH=64,
keep-ratio 0.7 -> k = 1433 selected tokens per batch row.

Sharding: 8 cores = (batch b in 0..3) x (query-half h in 0..1).  Each core
receives x[b].T with token columns rolled by h*1024 so that its own query
half always occupies columns 0..1023.

Key structure (on-device key AND query compaction):
 - x arrives as a bf16 hi/lo split so the router logits (4-pass bf16 matmul
   accumulated in fp32 PSUM) match fp32 within ~1e-6 -> exact top-k
   selection; all heavy matmuls run bf16.
 - The top-k threshold is found by 4 rounds of 128-ary refinement (emitted
   interleaved with the K/V-projection evictions so neither blocks the
   other on the in-order Vector stream); the selected token ids and their
   gate values are compacted with gpsimd sparse_gather (sentinel-padded via
   NaN-proof integer selects).
 - K,V (and Q) are projected token-major for all tokens (raw, ungated),
   stored to DRAM tables, and only the selected rows are fetched back with
   indirect DMA row-gathers (12x512B-row gathers for K|V, 6x2KB for my
   half's selected queries, zero rows behind the sentinel index).  K and Q
   are transposed to d-major with PE transpose-mode; gates are applied
   post-gather from the compacted gate lists.
 - Attention then runs over 12 key chunks x 768 query columns instead of
   16 x 1024: QK^T (zero-padded K=128 full-array bf16 matmuls -- sub-full-
   array matmuls run at half clock), exp on ScalarE (the bottleneck), att@V
   with the softmax denominator riding as a ones column of V.  Masked keys
   contribute exp(0)=1 to the reference softmax denominator, so a constant
   512 (= 615 masked keys - 103 sentinel rows) is added to the denominators;
   the sentinel query slot doubles as the shared output row of all masked
   queries (q=0 -> uniform attention), which the host broadcasts.
 - The host scatters the 768 compacted output rows back to their token
   positions using the device-computed index list (extra outputs idxq/nfq),
   falling back to a NumPy reference implementation if the selected-query
   count ever exceeded the padding (probability ~1e-6).
"""

import contextlib
import sys

import numpy as np

if "/opt/trn_rl_repo" not in sys.path:
    sys.path.insert(0, "/opt/trn_rl_repo")

import concourse.bass as bass  # noqa: F401
import concourse.bass_isa as bass_isa
import concourse.mybir as mybir
from concourse import bacc
from concourse.tile import TileContext

F32 = mybir.dt.float32
F32R = mybir.dt.float32r
BF16 = mybir.dt.bfloat16
I32 = mybir.dt.int32
I16 = mybir.dt.int16
U32 = mybir.dt.uint32
AF = mybir.ActivationFunctionType
ALU = mybir.AluOpType

B, N, C = 4, 2048, 1024
H, HKV, DH = 16, 4, 64
NQ = N // 2          # queries per core
KSEL = 1433          # max(1, int(N * 0.7))
NK2 = 1536           # padded selected-key count (12 chunks of 128)
KC2 = NK2 // 128     # 12
NQ2 = 768            # padded selected-query count (6 chunks of 128)
QC2 = NQ2 // 128     # 6
CC = C // 128        # contraction chunks (8)
QT_D = H * DH        # 1024
KV_D = HKV * DH      # 256
N_ROUNDS = 4         # topk threshold refinement rounds (8/128^4 = 3e-8)
LO0, W0 = -4.0, 8.0  # initial logit search interval (logit std ~0.65)
ONE_BITS = 0x3F800000  # fp32 bits of 1.0
INV_SQRT_DH = float(1.0 / np.sqrt(DH))


def _emit(nc, tc, ctx, io):
    xh, xl, wq, wkv, rw2, wo, out_d = (
        io["xh"], io["xl"], io["wq"], io["wkv"], io["rw2"], io["wo"],
        io["out"])

    # pools close LIFO: pa (refinement), pq (Q proj), pkv (KV proj), pg
    # (gather scratch), pm (masks), px (xh) -- created in reverse.
    const = ctx.enter_context(tc.tile_pool(name="const", bufs=1))
    small = ctx.enter_context(tc.tile_pool(name="small", bufs=1))
    big = ctx.enter_context(tc.tile_pool(name="big", bufs=1))
    dram = ctx.enter_context(tc.tile_pool(name="dram", bufs=1, space="DRAM"))

    px_ctx = contextlib.ExitStack()
    pm_ctx = contextlib.ExitStack()
    pg_ctx = contextlib.ExitStack()
    pkv_ctx = contextlib.ExitStack()
    pq_ctx = contextlib.ExitStack()
    pa_ctx = contextlib.ExitStack()
    px = px_ctx.enter_context(tc.tile_pool(name="px", bufs=1))
    psum1 = px_ctx.enter_context(tc.tile_pool(name="psum1", bufs=3, space="PSUM"))
    pm = pm_ctx.enter_context(tc.tile_pool(name="pm", bufs=1))
    pg = pg_ctx.enter_context(tc.tile_pool(name="pg", bufs=1))
    qtm_pool = pg_ctx.enter_context(tc.tile_pool(name="qtm", bufs=2))
    pkv = pkv_ctx.enter_context(tc.tile_pool(name="pkv", bufs=1))
    pq = pq_ctx.enter_context(tc.tile_pool(name="pq", bufs=1))
    pa = pa_ctx.enter_context(tc.tile_pool(name="pa", bufs=1))
    psum_r = pa_ctx.enter_context(tc.tile_pool(name="psum_r", bufs=1, space="PSUM"))

    # ---------------- constants ----------------
    ones2 = const.tile([2, 128], F32)
    nc.vector.memset(ones2, 1.0)
    iota128_i = const.tile([128, 1], I32)
    nc.gpsimd.iota(iota128_i, pattern=[[0, 1]], base=1, channel_multiplier=1)
    iota128 = const.tile([128, 1], F32)
    nc.vector.tensor_copy(iota128, iota128_i)
    sel8 = const.tile([16, CC, 128], F32R)
    nc.sync.dma_start(sel8, io["sel8"].bitcast(F32R))
    iota_w = const.tile([16, 128], I32)
    nc.gpsimd.iota(iota_w, pattern=[[16, 128]], base=0, channel_multiplier=1)
    iota_wf = const.tile([16, 128], F32)
    nc.vector.tensor_copy(iota_wf, iota_w)
    iota96 = const.tile([16, 96], I32)
    nc.gpsimd.iota(iota96, pattern=[[16, 96]], base=0, channel_multiplier=1)
    valid32 = const.tile([16, 96], I32)
    nc.vector.tensor_single_scalar(valid32, iota96, KSEL, op=ALU.is_lt)
    from concourse.masks import make_identity
    identity = const.tile([128, 128], BF16)
    make_identity(nc, identity)
    dum_out = const.tile([16, 1], F32)
    dum_nf = const.tile([1, 1], U32)
    # kt_z: zero-padded d-major K (slot j holds kv-heads 2j/2j+1; parity p
    # keeps rows [64p,64p+64) live, the partner half zero)
    kt_z = big.tile([128, 2, 2, NK2], BF16)
    nc.vector.memset(kt_z[0:64, 1, :, :], 0.0)
    nc.vector.memset(kt_z[64:128, 0, :, :], 0.0)
    # v_sb[t, kc, hk, 0:64]=V, col 64 = ones (softmax denominator), 65: zero
    v_sb = big.tile([128, KC2, HKV, 128], BF16)
    nc.vector.memset(v_sb[:, :, :, 64:128], 0.0)
    nc.vector.memset(v_sb[:, :, :, 64:65], 1.0)

    # ---------------- loads (split DMAs to spread across queues) ---------
    xh_sb = px.tile([128, CC, N], BF16)
    for cc in range(CC):
        for hf in range(2):
            nc.sync.dma_start(
                xh_sb[:, cc, hf * 1024:(hf + 1) * 1024],
                xh[cc * 128:(cc + 1) * 128, hf * 1024:(hf + 1) * 1024])
    rw_sb = pa.tile([128, CC, 128], BF16)
    for cc in range(CC):
        nc.sync.dma_start(rw_sb[:, cc, :], rw2[cc * 128:(cc + 1) * 128, :])
    xl_sb = pa.tile([128, CC, N], BF16)
    for cc in range(CC):
        for hf in range(2):
            nc.sync.dma_start(xl_sb[:, cc, hf * 1024:(hf + 1) * 1024],
                              xl[cc * 128:(cc + 1) * 128,
                                 hf * 1024:(hf + 1) * 1024])
    wkv_sb = pkv.tile([128, CC, 512], BF16)
    for cc in range(CC):
        nc.sync.dma_start(wkv_sb[:, cc, :], wkv[cc * 128:(cc + 1) * 128, :])
    wq_sb = pq.tile([128, CC, QT_D], BF16)
    for cc in range(CC):
        nc.sync.dma_start(wq_sb[:, cc, :], wq[cc * 128:(cc + 1) * 128, :])

    # ---------------- router: logits = (hi+lo) @ (rw_hi+rw_lo) ------------
    # M=128 stationary (126 zero cols) -> full-array matmuls stay at 2.4GHz
    rps = [psum_r.tile([128, 512], F32, tag=f"router_ps{g}",
                       name=f"router_ps{g}") for g in range(4)]
    for cc in range(CC):
        for g in range(4):
            nc.tensor.matmul(
                rps[g], rw_sb[:, cc, :], xh_sb[:, cc, g * 512:(g + 1) * 512],
                start=(cc == 0), stop=False)
    for cc in range(CC):
        for g in range(4):
            nc.tensor.matmul(
                rps[g], rw_sb[:, cc, :], xl_sb[:, cc, g * 512:(g + 1) * 512],
                start=False, stop=(cc == CC - 1))
    logits2 = pa.tile([2, N], F32)
    for g in range(4):
        nc.vector.tensor_copy(logits2[:, g * 512:(g + 1) * 512],
                              rps[g][0:2, :])
    lrep = pa.tile([128, N], F32)
    for g in range(4):
        ps = psum_r.tile([128, 512], F32, tag="bcast_ps")
        nc.tensor.matmul(ps, ones2, logits2[:, g * 512:(g + 1) * 512],
                         start=True, stop=True)
        nc.vector.tensor_copy(lrep[:, g * 512:(g + 1) * 512], ps)

    # ---------------- K/V projection (token-major, raw/ungated) ----------
    # kv_tm[t, i, 0:256]=K row, [256:512]=V row -> DRAM table (one DMA per
    # 128-token chunk to spread queues); the gate is applied post-gather.
    # The topk refinement rounds are EMITTED INTERLEAVED with the evictions
    # so the refinement's small DVE ops don't queue behind all 16 evicts on
    # the in-order Vector stream (nor the evicts behind the refinement).
    lo = small.tile([128, 1], F32)
    nc.vector.memset(lo, LO0)
    neg_edges = small.tile([128, 1], F32)
    acc = small.tile([128, 1], F32)
    sel = small.tile([128, 1], F32)
    ssum = small.tile([128, 1], F32)
    # the Sign output is never read: write it over xl (dead after router)
    sign_scr = xl_sb[:, 0, :]
    thr_acc = float(2 * KSEL - N)

    def emit_round(r):
        wstep = W0 / (128.0 ** (r + 1))
        nc.vector.scalar_tensor_tensor(
            neg_edges, iota128, -wstep, lo, op0=ALU.mult, op1=ALU.subtract)
        nc.scalar.activation(sign_scr, lrep, AF.Sign, bias=neg_edges,
                             scale=1.0, accum_out=acc)
        nc.vector.tensor_single_scalar(sel, acc, thr_acc, op=ALU.is_ge)
        nc.gpsimd.partition_all_reduce(ssum, sel, channels=128,
                                       reduce_op=bass_isa.ReduceOp.add)
        nc.vector.scalar_tensor_tensor(
            lo, ssum, wstep, lo, op0=ALU.mult, op1=ALU.add)

    kv_tm = pg.tile([128, 16, 512], BF16, tag="tm")
    kv_dram = dram.tile([N + 1, 512], BF16)
    zrow = pg.tile([1, 512], BF16)
    nc.vector.memset(zrow, 0.0)
    nc.sync.dma_start(kv_dram[N:N + 1, :], zrow)
    next_round = 0
    for i in range(16):
        ps = psum1.tile([128, 512], F32, tag="proj_ps", name=f"kv_ps{i}")
        for cc in range(CC):
            nc.tensor.matmul(
                ps, xh_sb[:, cc, i * 128:(i + 1) * 128], wkv_sb[:, cc, :],
                start=(cc == 0), stop=(cc == CC - 1))
        nc.vector.tensor_copy(kv_tm[:, i, :], ps)
        nc.sync.dma_start(
            kv_dram[i * 128:(i + 1) * 128, :].rearrange("(a p) c -> p a c",
                                                        p=128),
            kv_tm[:, i:i + 1, :])
        if i % 3 == 2 and next_round < N_ROUNDS:
            emit_round(next_round)
            next_round += 1
    while next_round < N_ROUNDS:
        emit_round(next_round)
        next_round += 1
    # dummy sparse_gather whose input depends on the last round: the
    # library load overlaps the m/cand step instead of running at t=0
    dum_in = pg.tile([16, 1], F32)
    nc.vector.scalar_tensor_tensor(dum_in, lo[0:16, :], -1.0,
                                   lo[0:16, :], op0=ALU.mult,
                                   op1=ALU.subtract)
    nc.gpsimd.sparse_gather(dum_out, dum_in, num_found=dum_nf)

    # m = (logit > lo) * sigmoid(logit); row 0 is enough for the compaction
    # sigmoid row parked in xl chunks 2-3 (dead after router, base 0)
    grep_row = xl_sb[0:1, 2:4, :].rearrange("p a b -> p (a b)").bitcast(F32)
    nc.scalar.activation(grep_row, lrep[0:1, :], AF.Sigmoid)
    m_row = logits2[0:1, :]
    nc.vector.scalar_tensor_tensor(
        m_row, lrep[0:1, :], lo[0:1, :], grep_row,
        op0=ALU.is_gt, op1=ALU.mult)

    # ---------------- compaction: index + gate lists ----------------
    m_dram = dram.tile([N], F32)
    nc.sync.dma_start(m_dram, m_row)
    m_w = pg.tile([16, 128], F32)
    nc.sync.dma_start(m_w, m_dram.rearrange("(f p) -> p f", p=16))
    selw = pg.tile([16, 128], F32)
    nc.vector.tensor_single_scalar(selw, m_w, 0.0, op=ALU.is_gt)
    # cand_idx = sel*(iota+1)-1 ; cand_gate = sel*(m+1)-1  (=-1 if unselected)
    cand_i = pg.tile([16, 128], F32)
    t1 = pg.tile([16, 128], F32)
    nc.vector.tensor_scalar(t1, iota_wf, 1.0, None, op0=ALU.add)
    nc.vector.tensor_tensor(cand_i, selw, t1, op=ALU.mult)
    nc.vector.tensor_scalar(cand_i, cand_i, 1.0, None, op0=ALU.subtract)
    cand_g = pg.tile([16, 128], F32)
    nc.vector.tensor_scalar(t1, m_w, 1.0, None, op0=ALU.add)
    nc.vector.tensor_tensor(cand_g, selw, t1, op=ALU.mult)
    nc.vector.tensor_scalar(cand_g, cand_g, 1.0, None, op0=ALU.subtract)
    # idx chain first: it alone gates the indirect gathers.  The gate chain
    # (second sparse_gather etc.) is only needed for the post-gather gate
    # multiplies and runs in parallel behind it.
    idx_f = pg.tile([16, 96], F32)
    nf = pg.tile([1, 1], U32)
    nc.gpsimd.sparse_gather(idx_f, cand_i, num_found=nf)
    # positions >= KSEL -> idx = sentinel N (zero row); integer selects so
    # sparse_gather tail garbage (even NaN bits) is annihilated
    idx32 = pg.tile([16, 96], I32)
    nc.vector.tensor_copy(idx32, idx_f)
    nc.vector.tensor_scalar(idx32, idx32, N, None, op0=ALU.subtract)
    nc.vector.tensor_tensor(idx32, valid32, idx32, op=ALU.mult)
    nc.vector.tensor_scalar(idx32, idx32, N, None, op0=ALU.add)
    idx_dram = dram.tile([NK2], I32)
    nc.sync.dma_start(idx_dram.rearrange("(f p) -> p f", p=16), idx32)
    idx_pc = pg.tile([128, KC2], I32)
    nc.sync.dma_start(idx_pc, idx_dram.rearrange("(c p) -> p c", p=128))

    # ---------------- Q projection (token-major, raw) -> DRAM table ------
    q_dram = dram.tile([NQ + 1, QT_D], BF16)
    for zh in range(2):
        nc.sync.dma_start(q_dram[NQ:NQ + 1, zh * 512:(zh + 1) * 512], zrow)
    for tt in range(8):
        q_tm = qtm_pool.tile([128, QT_D], BF16, tag="qtm", name=f"qtm{tt}")
        for qh in range(2):
            ps = psum1.tile([128, 512], F32, tag="proj_ps",
                            name=f"q_ps{tt}_{qh}")
            for cc in range(CC):
                nc.tensor.matmul(
                    ps, xh_sb[:, cc, tt * 128:(tt + 1) * 128],
                    wq_sb[:, cc, qh * 512:(qh + 1) * 512],
                    start=(cc == 0), stop=(cc == CC - 1))
            nc.vector.tensor_copy(q_tm[:, qh * 512:(qh + 1) * 512], ps)
        nc.sync.dma_start(
            q_dram[tt * 128:(tt + 1) * 128, :], q_tm)
    # ---------------- gather the selected K/V rows ----------------
    pa_ctx.close()  # frees the router PSUM banks for the transpose pool
    ptr_ctx = contextlib.ExitStack()
    ptr_psum = ptr_ctx.enter_context(
        tc.tile_pool(name="ptr", bufs=4, space="PSUM"))
    # remaining sparse_gathers grouped here: one gpsimd library window for
    # all four, then one reload for all 18 indirect gathers
    gat_f = pg.tile([16, 96], F32)
    nf2 = pg.tile([1, 1], U32)
    nc.gpsimd.sparse_gather(gat_f, cand_g, num_found=nf2)
    candq_i = pg.tile([16, 64], F32)
    candq_g = pg.tile([16, 64], F32)
    nc.vector.tensor_scalar(t1[:, 0:64], iota_wf[:, 0:64], 1.0, None,
                            op0=ALU.add)
    nc.vector.tensor_tensor(candq_i, selw[:, 0:64], t1[:, 0:64], op=ALU.mult)
    nc.vector.tensor_scalar(candq_i, candq_i, 1.0, None, op0=ALU.subtract)
    nc.vector.tensor_scalar(t1[:, 0:64], m_w[:, 0:64], 1.0, None, op0=ALU.add)
    nc.vector.tensor_tensor(candq_g, selw[:, 0:64], t1[:, 0:64], op=ALU.mult)
    nc.vector.tensor_scalar(candq_g, candq_g, 1.0, None, op0=ALU.subtract)
    idxq_f = pg.tile([16, 48], F32)
    nfq = pg.tile([1, 1], U32)
    nc.gpsimd.sparse_gather(idxq_f, candq_i, num_found=nfq)
    gatq_f = pg.tile([16, 48], F32)
    nfq2 = pg.tile([1, 1], U32)
    nc.gpsimd.sparse_gather(gatq_f, candq_g, num_found=nfq2)
    # q idx/gate ints + roundtrips (DVE/DMA only)
    idxq32 = pg.tile([16, 48], I32)
    nc.vector.tensor_copy(idxq32, idxq_f)
    nc.vector.tensor_scalar(idxq32, idxq32, NQ, 0, op0=ALU.min, op1=ALU.max)
    validq = pg.tile([16, 48], I32)
    nc.vector.tensor_single_scalar(validq, iota96[:, 0:48], NQ2 - 1,
                                   op=ALU.is_lt)
    nc.vector.tensor_scalar(idxq32, idxq32, NQ, None, op0=ALU.subtract)
    nc.vector.tensor_tensor(idxq32, validq, idxq32, op=ALU.mult)
    nc.vector.tensor_scalar(idxq32, idxq32, NQ, None, op0=ALU.add)
    gatq32 = pg.tile([16, 48], I32)
    nc.vector.tensor_copy(gatq32, gatq_f.bitcast(I32))
    nc.vector.tensor_scalar(gatq32, gatq32, ONE_BITS, None, op0=ALU.subtract)
    nc.vector.tensor_tensor(gatq32, validq, gatq32, op=ALU.mult)
    nc.vector.tensor_scalar(gatq32, gatq32, ONE_BITS, None, op0=ALU.add)
    idxq_dram = dram.tile([NQ2], I32)
    nc.sync.dma_start(idxq_dram.rearrange("(f p) -> p f", p=16), idxq32)
    gateq_dram = dram.tile([NQ2], F32)
    nc.sync.dma_start(gateq_dram.rearrange("(f p) -> p f", p=16),
                      gatq32.bitcast(F32))
    idxq_pc = pg.tile([128, QC2], I32)
    nc.sync.dma_start(idxq_pc, idxq_dram.rearrange("(c p) -> p c", p=128))
    gateq_row = pg.tile([1, NQ2], F32)
    nc.sync.dma_start(gateq_row, gateq_dram)
    nc.sync.dma_start(io["idxq"], idxq32)
    nc.sync.dma_start(io["nfq"], nfq.bitcast(I32))
    # all indirect gathers (single mlp-library window)
    kv_g = pg.tile([128, KC2, 512], BF16)
    for ch in range(KC2):
        nc.gpsimd.indirect_dma_start(
            out=kv_g[:, ch, :],
            out_offset=None,
            in_=kv_dram,
            in_offset=bass.IndirectOffsetOnAxis(ap=idx_pc[:, ch:ch + 1],
                                                axis=0),
        )
    q_g = pg.tile([128, 8, QT_D], BF16, tag="tm")
    for ch in range(QC2):
        nc.gpsimd.indirect_dma_start(
            out=q_g[:, ch, :],
            out_offset=None,
            in_=q_dram,
            in_offset=bass.IndirectOffsetOnAxis(ap=idxq_pc[:, ch:ch + 1],
                                                axis=0),
        )
    # k gate ints (DVE)
    gat32 = pg.tile([16, 96], I32)
    nc.vector.tensor_copy(gat32, gat_f.bitcast(I32))
    nc.vector.tensor_scalar(gat32, gat32, ONE_BITS, None, op0=ALU.subtract)
    nc.vector.tensor_tensor(gat32, valid32, gat32, op=ALU.mult)
    nc.vector.tensor_scalar(gat32, gat32, ONE_BITS, None, op0=ALU.add)
    gate_dram = dram.tile([NK2], F32)
    nc.sync.dma_start(gate_dram.rearrange("(f p) -> p f", p=16),
                      gat32.bitcast(F32))
    gate_g = pg.tile([128, KC2], F32)
    nc.sync.dma_start(gate_g, gate_dram.rearrange("(c p) -> p c", p=128))
    gate_row = pg.tile([1, NK2], F32)
    nc.sync.dma_start(gate_row, gate_dram)
    grep_sel = pg.tile([128, NK2], BF16, tag="gsel")
    for g in range(NK2 // 512):
        ps = psum1.tile([128, 512], F32, tag="proj_ps", name=f"gb{g}")
        nc.tensor.matmul(ps, ones2[0:1, :].bitcast(F32),
                         gate_row[:, g * 512:(g + 1) * 512],
                         start=True, stop=True)
        nc.vector.tensor_copy(grep_sel[:, g * 512:(g + 1) * 512], ps)
    # K -> d-major via PE transpose-mode (PE is idle here), split into the
    # kt_z parity halves
    for ch in range(KC2):
        for s in range(2):
            trp = ptr_psum.tile([128, 128], BF16, tag="trp",
                                name=f"trp{ch}_{s}")
            nc.tensor.transpose(trp, kv_g[:, ch, s * 128:(s + 1) * 128],
                                identity)
            cs = slice(ch * 128, (ch + 1) * 128)
            nc.vector.tensor_copy(kt_z[0:64, 0, s, cs], trp[0:64, :])
            nc.vector.tensor_copy(kt_z[64:128, 1, s, cs], trp[64:128, :])
    # gate: kt columns (d-major) and v rows (token-major)
    for par in range(2):
        for s in range(2):
            nc.vector.tensor_tensor(kt_z[:, par, s, :], kt_z[:, par, s, :],
                                    grep_sel, op=ALU.mult)
    for ch in range(KC2):
        nc.vector.tensor_scalar(
            v_sb[:, ch, :, 0:64],
            kv_g[:, ch, 256:512].rearrange("p (h e) -> p h e", e=64),
            gate_g[:, ch:ch + 1], None, op0=ALU.mult)
    qt_sb = big.tile([128, H // 2, NQ2], BF16)
    for ch in range(QC2):
        for sl8 in range(8):
            trp = ptr_psum.tile([128, 128], BF16, tag="trp",
                                name=f"trq{ch}_{sl8}")
            nc.tensor.transpose(
                trp, q_g[:, ch, sl8 * 128:(sl8 + 1) * 128], identity)
            nc.vector.tensor_copy(qt_sb[:, sl8, ch * 128:(ch + 1) * 128], trp)
    # gate the queries: qg_rep = broadcast(gateq_row) then qt *= qg_rep
    qg_rep = pg.tile([128, NQ2], BF16)
    for gi, (qs, qw) in enumerate([(0, 512), (512, 256)]):
        ps = psum1.tile([128, 512], F32, tag="proj_ps", name=f"qgb{gi}")
        nc.tensor.matmul(ps[:, 0:qw], ones2[0:1, :].bitcast(F32),
                         gateq_row[:, qs:qs + qw], start=True, stop=True)
        nc.vector.tensor_copy(qg_rep[:, qs:qs + qw], ps[:, 0:qw])
    for j in range(H // 2):
        nc.vector.tensor_tensor(qt_sb[:, j, :], qt_sb[:, j, :], qg_rep,
                                op=ALU.mult)

    ptr_ctx.close()
    pq_ctx.close()
    pkv_ctx.close()
    pg_ctx.close()
    pm_ctx.close()
    px_ctx.close()

    # ---------------- phase 2: attention ----------------
    ph2_ctx = contextlib.ExitStack()
    ph2 = ph2_ctx.enter_context(tc.tile_pool(name="ph2", bufs=1))
    wo_sb = ph2.tile([128, CC, C], BF16)
    for cc in range(CC):
        nc.sync.dma_start(wo_sb[:, cc, :], wo[cc * 128:(cc + 1) * 128, :])

    patt_ctx = contextlib.ExitStack()
    scr_pool = patt_ctx.enter_context(tc.tile_pool(name="scr_pool", bufs=2))
    p_pool = patt_ctx.enter_context(tc.tile_pool(name="p_pool", bufs=2))
    lg_pool = patt_ctx.enter_context(
        tc.tile_pool(name="lg_pool", bufs=2, space="PSUM"))
    att_pool = patt_ctx.enter_context(
        tc.tile_pool(name="att_pool", bufs=1, space="PSUM"))
    oT_sb = ph2.tile([128, CC, NQ2], BF16)
    denom_sb = ph2.tile([16, NQ2], F32)

    KQ = 2  # key chunks per P buffer
    QG = [(0, 512), (512, 256)]  # query column groups (NQ2 = 768)
    pair_heads = [(ha, ha + 4) for ha in (0, 1, 2, 3, 8, 9, 10, 11)]
    for hp, pair in enumerate(pair_heads):
        att_ps = [att_pool.tile([128, NQ2], F32, tag=f"att{m}", name=f"att{hp}_{m}")
                  for m in range(2)]
        pend = []
        for quarter in range(KC2 // KQ):
            p_t = p_pool.tile([128, KQ, 2 * NQ2], BF16, tag="p_t",
                              name=f"p_{hp}_{quarter}")
            for kci in range(KQ):
                kc = quarter * KQ + kci
                lg = [lg_pool.tile([128, NQ2], F32, tag="lg",
                                   name=f"lg{hp}_{kc}_{m2}") for m2 in range(2)]
                for qs, qw in QG:
                    for m in range(2):
                        h = pair[m]
                        hkv = h // 4
                        jq = (h % 4) + 4 * (h // 8)
                        nc.tensor.matmul(
                            lg[m][:, qs:qs + qw],
                            kt_z[:, hkv % 2, hkv // 2,
                                 kc * 128:(kc + 1) * 128],
                            qt_sb[:, jq, qs:qs + qw],
                            start=True, stop=True)
                for m in range(2):
                    nc.scalar.activation(
                        p_t[:, kci, m * NQ2:(m + 1) * NQ2], lg[m], AF.Exp,
                        scale=INV_SQRT_DH)
                for f in pend:
                    f()
                pend = []

                def attv(p_t=p_t, kci=kci, kc=kc):
                    for m in range(2):
                        hk = pair[m] // 4
                        for qs, qw in QG:
                            nc.tensor.matmul(
                                att_ps[m][:, qs:qs + qw],
                                v_sb[:, kc, hk, :],
                                p_t[:, kci, m * NQ2 + qs:m * NQ2 + qs + qw],
                                start=(kc == 0), stop=(kc == KC2 - 1))

                pend.append(attv)
        for f in pend:
            f()
        # fast evict: one [65, NQ2] copy frees the att psum for the next
        # pair; denom row DMA + bf16 conversion read the sbuf scratch.
        for m in range(2):
            h = pair[m]
            scr65 = scr_pool.tile([65, NQ2], F32R, tag="scr65",
                                  name=f"scr65_{hp}_{m}")
            nc.vector.tensor_copy(scr65, att_ps[m][0:65, :])
            nc.sync.dma_start(denom_sb[h:h + 1, :],
                              scr65[64:65, :].bitcast(F32))
            if h % 2 == 0:
                nc.vector.tensor_copy(oT_sb[0:64, h // 2, :], scr65[0:64, :])
            else:
                scr64 = scr_pool.tile([64, NQ2], BF16, tag="scr64",
                                      name=f"scr64_{hp}_{m}")
                nc.vector.tensor_copy(scr64, scr65[0:64, :])
                nc.sync.dma_start(oT_sb[64:128, h // 2, :], scr64)

    # denominators: add the 512 missing exp(0)=1 contributions (615 masked
    # keys in the reference minus 103 zero-padding sentinel rows on device),
    # then reciprocal + broadcast + scale
    nc.vector.tensor_scalar(denom_sb, denom_sb, 512.0, None, op0=ALU.add)
    rec16 = ph2.tile([16, NQ2], F32R)
    rec16_f = ph2.tile([16, NQ2], F32)
    with nc.allow_low_precision(reason="2e-5 rel err << output tolerance"):
        nc.vector.reciprocal_approx_fast(out=rec16_f, in_=denom_sb)
    nc.vector.tensor_copy(rec16, rec16_f)
    ph3_ctx = contextlib.ExitStack()
    out_pool = ph3_ctx.enter_context(tc.tile_pool(name="out_pool", bufs=2))
    for gi, (qs, qw) in enumerate([(0, 512), (512, 256)]):
        for dd in range(CC):
            bps = lg_pool.tile([128, 512], F32, tag="lg", name=f"bps{dd}_{gi}")
            nc.tensor.matmul(
                bps[:, 0:qw], sel8[:, dd, :], rec16[:, qs:qs + qw],
                start=True, stop=True)
            nc.vector.tensor_tensor(
                oT_sb[:, dd, qs:qs + qw], oT_sb[:, dd, qs:qs + qw],
                bps[:, 0:qw], op=ALU.mult)
        for tt in range(qs // 128, (qs + qw) // 128):
            out_sb = out_pool.tile([128, C], F32, tag="out_sb",
                                   name=f"out_sb{tt}")
            for og in range(C // 512):
                ps = att_pool.tile([128, 512], F32, tag=f"att{og}",
                                   name=f"out_ps{tt}_{og}")
                for dd in range(CC):
                    nc.tensor.matmul(
                        ps, oT_sb[:, dd, tt * 128:(tt + 1) * 128],
                        wo_sb[:, dd, og * 512:(og + 1) * 512],
                        start=(dd == 0), stop=(dd == CC - 1))
                nc.scalar.copy(out_sb[:, og * 512:(og + 1) * 512], ps)
            for hf in range(2):
                nc.sync.dma_start(
                    out_d[tt * 128:(tt + 1) * 128, hf * 512:(hf + 1) * 512],
                    out_sb[:, hf * 512:(hf + 1) * 512])
    ph3_ctx.close()
    patt_ctx.close()
    ph2_ctx.close()


_NC = None


def build_program():
    global _NC
    if _NC is not None:
        return _NC
    from contextlib import ExitStack

    nc = bacc.Bacc("TRN2", target_bir_lowering=False, debug=False, num_devices=8)
    io = {
        "xh": nc.dram_tensor("xh", (C, N), BF16, kind="ExternalInput").ap(),
        "xl": nc.dram_tensor("xl", (C, N), BF16, kind="ExternalInput").ap(),
        "wq": nc.dram_tensor("wq", (C, QT_D), BF16, kind="ExternalInput").ap(),
        "wkv": nc.dram_tensor("wkv", (C, 512), BF16, kind="ExternalInput").ap(),
        "rw2": nc.dram_tensor("rw2", (C, 128), BF16,
                              kind="ExternalInput").ap(),
        "wo": nc.dram_tensor("wo", (C, C), BF16, kind="ExternalInput").ap(),
        "sel8": nc.dram_tensor("sel8", (16, CC, 128), F32,
                               kind="ExternalInput").ap(),
        "out": nc.dram_tensor("out", (NQ2, C), F32,
                              kind="ExternalOutput").ap(),
        "idxq": nc.dram_tensor("idxq", (16, 48), I32,
                               kind="ExternalOutput").ap(),
        "nfq": nc.dram_tensor("nfq", (1,), I32, kind="ExternalOutput").ap(),
    }
    with TileContext(nc) as tc:
        with ExitStack() as ctx:
            _emit(nc, tc, ctx, io)
    nc.compile()
    _NC = nc
    return nc


def _permute_wq(wq):
    """Column-permute wq so QT slot j's 128 cols = heads (ha, ha+4) contig."""
    wq = np.asarray(wq, np.float32).reshape(C, H, DH)
    order = []
    for j in range(H // 2):
        ha = j if j < 4 else j + 4
        order += [ha, ha + 4]
    return np.ascontiguousarray(wq[:, order, :].reshape(C, H * DH))


def make_in_maps(x, router_w, wq, wk, wv, wo):
    import ml_dtypes

    bf16 = ml_dtypes.bfloat16
    wq = _permute_wq(wq).astype(bf16)
    rw = np.asarray(router_w, np.float32)
    rw_hi = rw.astype(bf16)
    rw_lo = (rw - rw_hi.astype(np.float32)).astype(bf16)
    rw2 = np.zeros((C, 128), np.float32).astype(bf16)
    rw2[:, 0:1] = rw_hi
    rw2[:, 1:2] = rw_lo
    rw2 = np.ascontiguousarray(rw2)
    wkv = np.ascontiguousarray(
        np.concatenate([np.asarray(wk, np.float32),
                        np.asarray(wv, np.float32)], axis=1).astype(bf16))
    wo_b = np.asarray(wo, np.float32).astype(bf16)
    sel8 = np.zeros((16, CC, 128), np.float32)
    for dd in range(CC):
        for p in range(128):
            sel8[2 * dd + p // 64, dd, p] = 1.0
    in_maps = []
    for core in range(8):
        b, h = core // 2, core % 2
        xT_core = np.ascontiguousarray(
            np.roll(np.asarray(x[b], np.float32).T, -h * NQ, axis=1))
        xh_core = xT_core.astype(bf16)
        xl_core = (xT_core - xh_core.astype(np.float32)).astype(bf16)
        in_maps.append({
            "xh": np.ascontiguousarray(xh_core),
            "xl": np.ascontiguousarray(xl_core),
            "sel8": sel8,
            "wq": np.ascontiguousarray(wq),
            "wkv": wkv,
            "rw2": rw2,
            "wo": np.ascontiguousarray(wo_b),
        })
    return in_maps


def _numpy_fallback(x, router_w, router_b, wq, bq, wk, bk, wv, bv, wo, bo):
    x = np.asarray(x, np.float32)
    gate = 1.0 / (1.0 + np.exp(-(x @ router_w + router_b)))
    xg = x * gate
    scores = gate[..., 0]
    idx = np.argsort(-scores, axis=-1, kind="stable")[:, :KSEL]
    mask = np.zeros((x.shape[0], x.shape[1]), np.float32)
    np.put_along_axis(mask, idx, 1.0, axis=1)
    xg = xg * mask[..., None]
    q = (xg @ wq + bq).reshape(B, N, H, DH)
    kk = np.repeat((xg @ wk + bk).reshape(B, N, HKV, DH), H // HKV, axis=2)
    v = np.repeat((xg @ wv + bv).reshape(B, N, HKV, DH), H // HKV, axis=2)
    att = np.einsum("bqhd,bkhd->bhqk", q, kk) / np.float32(np.sqrt(DH))
    att = att - att.max(-1, keepdims=True)
    att = np.exp(att)
    att = att / att.sum(-1, keepdims=True)
    o = np.einsum("bhqk,bkhd->bqhd", att, v).reshape(B, N, C)
    return (o @ wo + bo).astype(np.float32)


def kernel(x, router_w, router_b, wq, bq, wk, bk, wv, bv, wo, bo):
    x = np.asarray(x)
    biases = [router_b, bq, bk, bv, bo]
    if any(float(np.abs(np.asarray(t)).max()) != 0.0 for t in biases):
        return _numpy_fallback(x, router_w, router_b, wq, bq, wk, bk, wv, bv,
                               wo, bo)

    from concourse import bass_utils

    nc = build_program()
    in_maps = make_in_maps(x, router_w, wq, wk, wv, wo)
    res = bass_utils.run_bass_kernel_spmd(nc, in_maps, core_ids=list(range(8)))
    out = np.empty((B, N, C), np.float32)
    for core in range(8):
        b, h = core // 2, core % 2
        r = res.results[core]
        nf = int(np.asarray(r["nfq"]).view(np.int32)[0])
        idxq = np.ascontiguousarray(np.asarray(r["idxq"]).T).reshape(-1)
        rows = np.asarray(r["out"])
        if not (0 < nf <= NQ2 - 1):
            return _numpy_fallback(x, router_w, router_b, wq, bq, wk, bk,
                                   wv, bv, wo, bo)
        ids = idxq[:nf]
        if ids.min() < 0 or ids.max() >= NQ:
            return _numpy_fallback(x, router_w, router_b, wq, bq, wk, bk,
                                   wv, bv, wo, bo)
        # non-selected queries all share the sentinel (zero-q) row's output
        block = np.repeat(rows[NQ2 - 1:NQ2, :], NQ, axis=0)
        block[ids] = rows[:nf]
        out[b, h * NQ:(h + 1) * NQ, :] = block
    return out
